# revision 21
# baseline (speedup 1.0000x reference)
"""Trainium2 Bass kernel for BalSupMoCoNet supervised-contrastive loss.

Math (equivalent restructuring of the reference, validated to ~4e-7 rel):
  all_feature f = [l2norm(q); ba_queue; nonba_queue]   (N x 128, N = 16640)
  sim_ij = f_i . f_j / T,  C = 1/T  (since |f_i.f_j| <= 1, C is a stable shift
  and the log-sum-exp shift cancels exactly in log-prob)
  S1_i   = sum_{j != i} exp(sim_ij - C) = [sum_j exp(f_i.f_j/T - C)] - 1.0
  g_l    = sum_{j: label_j = l} f_j      (two 128-vectors)
  S2_i   = sum_{j: label_j = label_i, j != i} sim_ij = (f_i.g_{label_i} - 1)/T
  contrib_i = S2_i / P_i - C - log(S1_i),   P_i = count(label_i) - 1
  loss   = -(1/N) sum_i contrib_i

Per core (8-way row-parallel SPMD): 2080 rows, padded to 17 row-tiles of 128.
Per row-tile: 33 PE matmuls (f32, 128-contraction) into PSUM, ScalarE exp
in-place on PSUM with fused accum_out row-sums (the O(N^2) hot path), tiny
DVE epilogue. g0/g1 ride along as 2 extra rhs columns in the tail matmul.
"""

import sys

import numpy as np

try:
    import concourse.bass as bass
except ImportError:
    sys.path.insert(0, "/opt/trn_rl_repo")
    import concourse.bass as bass

import concourse.bacc as bacc
import concourse.tile as tile
from concourse import mybir
from concourse.bass_utils import run_bass_kernel_spmd

AF = mybir.ActivationFunctionType
ALU = mybir.AluOpType
F32 = mybir.dt.float32
F32R = mybir.dt.float32r
BF16 = mybir.dt.bfloat16
TS = bass.ts


class Cfg:
    def __init__(self, B=256, K=8192, ncores=8, CGW=2048, MMW=512):
        self.B, self.K, self.D, self.ncores = B, K, 128, ncores
        self.T = 0.07
        self.C = 1.0 / self.T
        self.N = B + 2 * K
        assert self.N % ncores == 0
        self.RPC = self.N // ncores              # rows per core
        self.RT = -(-self.RPC // 128)            # row tiles per core
        self.RSLOTS = self.RT * 128
        assert K % CGW == 0 and CGW % MMW == 0
        self.CGW, self.MMW = CGW, MMW
        self.NCH = K // CGW                      # chunks per queue
        self.MPC = CGW // MMW                    # matmuls per chunk
        self.QT = -(-B // 128)                   # q row tiles
        self.GPR = 2 * self.NCH + 1              # accum groups per row tile
        self.TAILW = B + 2                       # q cols + 2 g cols


FULL = Cfg()


def build_program(cfg):
    """Build the SPMD Bass program. Returns nc."""
    nc = bacc.Bacc("TRN2", target_bir_lowering=False, debug=False,
                   enable_asserts=True, num_devices=cfg.ncores)

    B, K, D, T, C = cfg.B, cfg.K, cfg.D, cfg.T, cfg.C
    CGW, MMW, NCH, MPC = cfg.CGW, cfg.MMW, cfg.NCH, cfg.MPC
    RT, QT, GPR = cfg.RT, cfg.QT, cfg.GPR

    d_q = nc.dram_tensor("q", [B, D], F32, kind="ExternalInput").ap()
    d_ba = nc.dram_tensor("baT", [D, K], BF16, kind="ExternalInput").ap()
    d_nb = nc.dram_tensor("nbT", [D, K], BF16, kind="ExternalInput").ap()
    d_rows = nc.dram_tensor("rowsT", [D, cfg.RSLOTS], BF16, kind="ExternalInput").ap()
    d_ind2 = nc.dram_tensor("ind2", [B, 2], F32, kind="ExternalInput").ap()
    d_mi = nc.dram_tensor("m_ind1", [128, RT], F32, kind="ExternalInput").ap()
    d_mr = nc.dram_tensor("m_rs", [128, RT], F32, kind="ExternalInput").ap()
    d_mw = nc.dram_tensor("m_w", [128, RT], F32, kind="ExternalInput").ap()
    d_mdg = nc.dram_tensor("m_dg", [128, RT], F32, kind="ExternalInput").ap()
    d_sv = nc.dram_tensor("svec", [128, 1], F32, kind="ExternalInput").ap()
    d_id = nc.dram_tensor("ident", [128, 128], F32, kind="ExternalInput").ap()
    d_out = nc.dram_tensor("out", [128, 1], F32, kind="ExternalOutput").ap()

    from contextlib import ExitStack
    with tile.TileContext(nc) as tc, ExitStack() as ctx:
        feat = ctx.enter_context(tc.tile_pool(name="feat", bufs=1))
        consts = ctx.enter_context(tc.tile_pool(name="consts", bufs=1))
        work = ctx.enter_context(tc.tile_pool(name="work", bufs=2))
        accs = ctx.enter_context(tc.tile_pool(name="accs", bufs=1))
        pspool = ctx.enter_context(tc.tile_pool(name="psum", bufs=2, space="PSUM"))

        # ---- q + ind2 first (tiny): Square/Sqrt precede the first Exp in ACT
        # program order, so their input must not wait behind bulk DMA
        qrows = []
        ind2_tiles = []
        for t in range(QT):
            qp = min(128, B - t * 128)
            qrow = work.tile([qp, D], F32, tag="qrow")
            nc.sync.dma_start(out=qrow[:], in_=d_q[t * 128:t * 128 + qp, :])
            qrows.append((qrow, qp))
            i2 = work.tile([qp, 2], F32, tag=f"ind2_{t}")
            nc.sync.dma_start(out=i2[:], in_=d_ind2[t * 128:t * 128 + qp, :])
            ind2_tiles.append(i2)

        # ---- per-core row block: the q-normalize/patch chain and the
        # first matmuls depend on it, while late chunks only gate late groups
        rowsA_w = min(cfg.RSLOTS, QT * 128)
        rowsA = feat.tile([D, rowsA_w], BF16, tag="rowsA")
        nc.sync.dma_start(out=rowsA[:], in_=d_rows[:, 0:rowsA_w])
        rowsB = None
        if cfg.RSLOTS > rowsA_w:
            rowsB = feat.tile([D, cfg.RSLOTS - rowsA_w], BF16, tag="rowsB")
            nc.gpsimd.dma_start(out=rowsB[:], in_=d_rows[:, rowsA_w:cfg.RSLOTS])
        qg = feat.tile([D, cfg.TAILW], BF16, tag="qg")   # qn^T cols + g0,g1

        ident = consts.tile([128, 128], F32, tag="ident")
        nc.sync.dma_start(out=ident[:], in_=d_id[:])
        svec = consts.tile([128, 1], F32, tag="svec")
        nc.sync.dma_start(out=svec[:], in_=d_sv[:])
        m_i1 = consts.tile([128, RT], F32, tag="mi")
        nc.sync.dma_start(out=m_i1[:], in_=d_mi[:])
        m_rs = consts.tile([128, RT], F32, tag="mr")
        nc.sync.dma_start(out=m_rs[:], in_=d_mr[:])
        m_w = consts.tile([128, RT], F32, tag="mw")
        nc.sync.dma_start(out=m_w[:], in_=d_mw[:])
        m_dg = consts.tile([128, RT], F32, tag="mdg")
        nc.sync.dma_start(out=m_dg[:], in_=d_mdg[:])
        # per-partition bias vectors for ACTIVATE (only 0.0/1.0 are built in)
        b_negC = consts.tile([128, 1], F32, tag="b_negC")
        nc.vector.memset(b_negC[:], -C)
        b_neg1 = consts.tile([128, 1], F32, tag="b_neg1")
        nc.vector.memset(b_neg1[:], -1.0)

        # ---- shared feature chunks (transposed layout [D, cols]) ----
        chunks = []                              # 2*NCH rhs chunk tiles
        for i in range(NCH):
            t = feat.tile([D, CGW], BF16, tag=f"ba{i}")
            nc.sync.dma_start(out=t[:], in_=d_ba[:, TS(i, CGW)])
            chunks.append(t)
        for i in range(NCH):
            t = feat.tile([D, CGW], BF16, tag=f"nb{i}")
            nc.gpsimd.dma_start(out=t[:], in_=d_nb[:, TS(i, CGW)])
            chunks.append(t)

        # ---- normalize q on device, transpose into qg, patch rowsA ----
        qn_tiles = []
        for t in range(QT):
            qrow, qp = qrows[t]
            sqscr = work.tile([qp, D], F32, tag="sqscr")
            nc.vector.tensor_mul(sqscr[:], qrow[:], qrow[:])
            nrm2 = work.tile([qp, 1], F32, tag="nrm2")
            nc.vector.reduce_sum(nrm2[:], sqscr[:], axis=mybir.AxisListType.X)
            lnn = work.tile([qp, 1], F32, tag="lnn")
            nc.scalar.activation(lnn[:], nrm2[:], AF.Ln)
            rinv = work.tile([qp, 1], F32, tag="rinv")
            nc.scalar.activation(rinv[:], lnn[:], AF.Exp, scale=-0.5)
            qn = work.tile([qp, D], F32, tag="qn")
            nc.vector.tensor_scalar_mul(qn[:], qrow[:], rinv[:])
            qn_tiles.append((qn, qp))
            psq = pspool.tile([128, CGW], F32, tag="ps")
            nc.tensor.transpose(psq[:, 0:qp], qn[:], ident[0:qp, 0:qp])
            # qg q-columns = qn^T
            nc.vector.tensor_copy(qg[:, t * 128:t * 128 + qp], psq[:, 0:qp])
            # patch this core's own-row block: rowsA = s*qnT + (1-s)*rowsA
            dscr = work.tile([D, qp], F32, tag="dscr")
            nc.vector.tensor_sub(dscr[:], psq[:, 0:qp], rowsA[:, t * 128:t * 128 + qp])
            nc.vector.scalar_tensor_tensor(
                rowsA[:, t * 128:t * 128 + qp], dscr[:], svec[:],
                rowsA[:, t * 128:t * 128 + qp], op0=ALU.mult, op1=ALU.add)

        # ---- g vectors: q part via PE, queue parts via DVE reduces ----
        psg = pspool.tile([128, CGW], F32, tag="ps")
        for t, (qn, qp) in enumerate(qn_tiles):
            nc.tensor.matmul(psg[:, 0:2], qn[:], ind2_tiles[t][:],
                             start=(t == 0), stop=(t == len(qn_tiles) - 1))
        gsb = accs.tile([D, 2 * NCH], F32, tag="gsb")
        for i in range(2 * NCH):
            nc.vector.reduce_sum(gsb[:, i:i + 1], chunks[i][:],
                                 axis=mybir.AxisListType.X)
        batot = accs.tile([D, 1], F32, tag="batot")
        nc.vector.reduce_sum(batot[:], gsb[:, 0:NCH], axis=mybir.AxisListType.X)
        nbtot = accs.tile([D, 1], F32, tag="nbtot")
        nc.vector.reduce_sum(nbtot[:], gsb[:, NCH:2 * NCH], axis=mybir.AxisListType.X)
        # g0 = q-part(label 0) + nonba sum; g1 = q-part(label 1) + ba sum
        nc.vector.tensor_add(qg[:, B:B + 1], psg[:, 0:1], nbtot[:])
        nc.vector.tensor_add(qg[:, B + 1:B + 2], psg[:, 1:2], batot[:])

        # ---- main loop: sim matmuls + fused exp/rowsum ----
        sums = accs.tile([128, RT * GPR], F32, tag="sums")
        dg0 = accs.tile([128, RT], F32, tag="dg0")
        dg1 = accs.tile([128, RT], F32, tag="dg1")
        def lhsT_of(rt):
            if rt < QT:
                return rowsA[:, TS(rt, 128)]
            return rowsB[:, TS(rt - QT, 128)]

        # process DMA-only row tiles first: tiles < QT wait on the whole
        # q-normalize/patch chain, so doing them last hides that latency
        rt_order = [rt for rt in range(RT) if rt >= QT] + list(range(min(QT, RT)))
        for rt in rt_order:
            lhsT = lhsT_of(rt)
            for gi in range(2 * NCH):
                ps = pspool.tile([128, CGW], F32, tag="ps")
                for j in range(MPC):
                    nc.tensor.matmul(ps[:, TS(j, MMW)], lhsT,
                                     chunks[gi][:, TS(j, MMW)],
                                     start=True, stop=True)
                nc.scalar.activation(ps[:], ps[:], AF.Exp, bias=b_negC[:], scale=1.0 / T,
                                     accum_out=sums[:, rt * GPR + gi:rt * GPR + gi + 1])
        # tail phase: small q+g groups batched after the uniform main rhythm
        for rt in rt_order:
            ps = pspool.tile([128, CGW], F32, tag="ps")
            nc.tensor.matmul(ps[:, 0:cfg.TAILW], lhsT_of(rt), qg[:],
                             start=True, stop=True)
            nc.scalar.activation(ps[:, 0:B], ps[:, 0:B], AF.Exp, bias=b_negC[:],
                                 scale=1.0 / T,
                                 accum_out=sums[:, rt * GPR + 2 * NCH:rt * GPR + 2 * NCH + 1])
            nc.vector.tensor_copy(dg0[:, rt:rt + 1], ps[:, B:B + 1])
            nc.vector.tensor_copy(dg1[:, rt:rt + 1], ps[:, B + 1:B + 2])

        # ---- epilogue (all [128, RT] vector math) ----
        red = accs.tile([128, RT], F32, tag="red")
        nc.vector.reduce_sum(red[:], sums[:].rearrange("p (t g) -> p t g", g=GPR),
                             axis=mybir.AxisListType.X)
        expdg = accs.tile([128, RT], F32, tag="expdg")
        nc.scalar.activation(expdg[:], m_dg[:], AF.Exp, bias=b_negC[:],
                             scale=1.0 / T)
        nc.vector.tensor_sub(red[:], red[:], expdg[:])            # S1
        lg = accs.tile([128, RT], F32, tag="lg")
        nc.scalar.activation(lg[:], red[:], AF.Ln)                # log(S1)
        e1 = accs.tile([128, RT], F32, tag="e1")
        nc.vector.tensor_sub(e1[:], dg1[:], dg0[:])
        nc.vector.tensor_mul(e1[:], e1[:], m_i1[:])
        nc.vector.tensor_add(e1[:], e1[:], dg0[:])            # dot(f_i, g_label)
        nc.vector.tensor_sub(e1[:], e1[:], m_dg[:])
        nc.vector.tensor_mul(e1[:], e1[:], m_rs[:])           # S2/P (rs = 1/(T*P))
        nc.vector.tensor_scalar_add(e1[:], e1[:], -C)
        nc.vector.tensor_sub(e1[:], e1[:], lg[:])             # contrib
        nc.vector.tensor_mul(e1[:], e1[:], m_w[:])            # * (-1/N) or 0
        outv = accs.tile([128, 1], F32, tag="outv")
        nc.vector.reduce_sum(outv[:], e1[:], axis=mybir.AxisListType.X)
        nc.sync.dma_start(out=d_out[:], in_=outv[:])

    nc.compile()
    return nc


def prep_in_maps(cfg, q, ba_queue, nonba_queue, targets):
    q = np.ascontiguousarray(np.asarray(q), dtype=np.float32)
    ba = np.asarray(ba_queue, dtype=np.float32)
    nb = np.asarray(nonba_queue, dtype=np.float32)
    tg = np.asarray(targets).astype(np.int64)
    B, K, N = cfg.B, cfg.K, cfg.N

    import ml_dtypes
    BF = ml_dtypes.bfloat16

    qT = np.ascontiguousarray(q.T)
    baT = np.ascontiguousarray(ba.T)
    nbT = np.ascontiguousarray(nb.T)
    fullT = np.concatenate([qT, baT, nbT], axis=1)          # [128, N] (q unnormalized)
    fullT_bf = fullT.astype(BF)

    # exact squared norms of the bf16 features the matmul will see; for q
    # columns use bf16(host-normalized q) — matches device qn to ~1e-6
    qn_host = q / np.clip(np.linalg.norm(q, axis=1, keepdims=True), 1e-12, None)
    fq_norm = np.concatenate([qn_host.T, baT, nbT], axis=1).astype(BF).astype(np.float32)
    dvec_full = (fq_norm * fq_norm).sum(axis=0).astype(np.float32)      # [N]
    pad_col_bf = baT[:, 0:1].astype(BF)
    pad_dg = float((pad_col_bf.astype(np.float32) ** 2).sum())

    labels = np.concatenate([tg, np.ones(K, np.int64), np.zeros(K, np.int64)])
    c1 = int(labels.sum())
    c0 = N - c1
    P = np.where(labels == 1, c1 - 1, c0 - 1).astype(np.float64)
    rs_full = (1.0 / (cfg.T * P)).astype(np.float32)
    ind1_full = labels.astype(np.float32)
    w_full = np.full(N, -1.0 / N, dtype=np.float32)
    ind2 = np.stack([1.0 - tg, tg], axis=1).astype(np.float32)
    ident = np.eye(128, dtype=np.float32)

    def per_tile_layout(vec_core, fill=0.0):
        padded = np.full(cfg.RSLOTS, fill, dtype=np.float32)
        padded[:cfg.RPC] = vec_core
        return np.ascontiguousarray(padded.reshape(cfg.RT, 128).T)

    in_maps = []
    for c in range(cfg.ncores):
        lo = c * cfg.RPC
        rows = fullT_bf[:, lo:lo + cfg.RPC]
        if cfg.RSLOTS > cfg.RPC:
            # pad with a normalized queue row (raw q cols can have norm ~11,
            # which would overflow exp via the pad row's self-dot)
            pad = np.repeat(pad_col_bf, cfg.RSLOTS - cfg.RPC, axis=1)
            rows = np.concatenate([rows, pad], axis=1)
        in_maps.append({
            "q": q,
            "baT": np.ascontiguousarray(baT.astype(BF)),
            "nbT": np.ascontiguousarray(nbT.astype(BF)),
            "rowsT": np.ascontiguousarray(rows),
            "ind2": ind2,
            "m_ind1": per_tile_layout(ind1_full[lo:lo + cfg.RPC]),
            "m_rs": per_tile_layout(rs_full[lo:lo + cfg.RPC]),
            "m_w": per_tile_layout(w_full[lo:lo + cfg.RPC]),
            # pad rows need their true bf16 norm^2: a wrong diag makes
            # S1 <= 0 -> log NaN -> NaN * 0 weight still poisons the sum
            "m_dg": per_tile_layout(dvec_full[lo:lo + cfg.RPC], fill=pad_dg),
            "svec": np.full((128, 1), 1.0 if c == 0 else 0.0, dtype=np.float32),
            "ident": ident,
        })
    return in_maps


_PROGRAM = None


def get_program():
    global _PROGRAM
    if _PROGRAM is None:
        _PROGRAM = build_program(FULL)
    return _PROGRAM


def run_on_hw(in_maps, trace=False):
    nc = get_program()
    return run_bass_kernel_spmd(nc, in_maps, list(range(FULL.ncores)), trace=trace)


def kernel(q, ba_queue, nonba_queue, targets):
    in_maps = prep_in_maps(FULL, q, ba_queue, nonba_queue, targets)
    res = run_on_hw(in_maps)
    total = sum(float(r["out"].astype(np.float64).sum()) for r in res.results)
    return np.array(total, dtype=np.float32)
